# revision 1
# baseline (speedup 1.0000x reference)
"""Trainium2 Bass kernel for the 200-layer 1-channel Conv1d(k=7) chain + FC + sigmoid.

Strategy (pure data parallel, 8 cores, batch 1024 -> 128/core):
  - On-chip layout keeps the sequence dim on SBUF partitions, interleaved mod 128:
      H[p, 128*c + b] = h[b, 128*c + p]
    so each conv layer is a banded matmul contracting over partitions.
  - The 7-tap band is expressed per layer as a [64, 32] stacked weight block
    (D = within-32-group band, E = cross-group band).  Each 512-wide PSUM block
    is computed by 5 concurrent PE matmuls on disjoint 32x32 sub-array tiles:
      3x (K=64 combined D+E), 1x (K=32 D at row-group 3), 1x (K=6 column-wrap).
  - float32r matmuls (full PE rate at N>=256); relu+bias applied on the
    PSUM->SBUF copy, alternating between the Scalar (ACT) and Vector (DVE)
    engines so neither becomes the serial bottleneck.
  - x is DMA'd naturally and transposed on-chip through the PE (f32).
  - Final Linear(188->91) runs as two accumulating matmuls + fused Sigmoid.
"""

import os
import sys

if "/opt/trn_rl_repo" not in sys.path:
    sys.path.insert(0, "/opt/trn_rl_repo")

from contextlib import ExitStack

import numpy as np

import concourse.bacc as bacc
import concourse.bass as bass
import concourse.mybir as mybir
from concourse import tile
from concourse.bass_utils import run_bass_kernel_spmd

N_CORES = 8
BC = 128          # batch per core
L0 = 1388
N_LAYERS = 200
K7 = 7
FC_IN = 188
FC_OUT = 91

F32 = mybir.dt.float32
F32R = mybir.dt.float32r
BF16 = mybir.dt.bfloat16
AFT = mybir.ActivationFunctionType
ALU = mybir.AluOpType

MODE = "packed32"         # "packed64" | "packed32" | "simple"
BAND_CHUNKS = 4           # weight DMA prefetch chunks (50 layers each)


def _make_bands(conv_w: np.ndarray) -> np.ndarray:
    """[128, 200*64] f32.  Layer l occupies free cols [64l, 64l+64):
    cols [64l, 64l+32)  = D (D[j,r]=w[j-r]) replicated in all four
                          32-partition groups (weights must share the rhs
                          base partition);
    cols [64l+32, 64l+64) = E (E[j,r]=w[32+j-r], rows 0:6) replicated at
                          partition bases 0/32/64/96."""
    out = np.zeros((128, N_LAYERS * 64), np.float32)
    j = np.arange(32)[:, None]
    r = np.arange(32)[None, :]
    dd = j - r            # D taps at 0..6
    ee = 32 + j - r       # E taps at 0..6
    for l in range(N_LAYERS):
        w = conv_w[l]
        D = np.where((dd >= 0) & (dd <= 6), w[np.clip(dd, 0, 6)], 0.0)
        E = np.where((ee >= 0) & (ee <= 6), w[np.clip(ee, 0, 6)], 0.0)
        fo = 64 * l
        for g in range(4):
            out[32 * g:32 * g + 32, fo:fo + 32] = D
            out[32 * g:32 * g + 6, fo + 32:fo + 64] = E[0:6]
    return out


def _make_bands_full(conv_w: np.ndarray) -> np.ndarray:
    """[128, 200*128] f32: per layer the full 128x128 within-column band."""
    out = np.zeros((128, N_LAYERS * 128), np.float32)
    j = np.arange(128)[:, None]
    r = np.arange(128)[None, :]
    dd = j - r
    for l in range(N_LAYERS):
        w = conv_w[l]
        out[:, 128 * l:128 * (l + 1)] = np.where(
            (dd >= 0) & (dd <= 6), w[np.clip(dd, 0, 6)], 0.0)
    return out




def _make_bands2(conv_w: np.ndarray) -> np.ndarray:
    """[128, 200*256]: cols [256l,256l+128) full within-column band
    (B[j,r]=w[j-r], 0<=j-r<=6); cols [256l+128,256l+256) rows 0:6 wrap
    (W[j,r]=w[128+j-r], nonzero r>=122)."""
    out = np.zeros((128, N_LAYERS * 256), np.float32)
    j = np.arange(128)[:, None]
    r = np.arange(128)[None, :]
    dd = j - r
    j6 = np.arange(6)[:, None]
    ww = 128 + j6 - r
    for l in range(N_LAYERS):
        w = conv_w[l]
        fo = 256 * l
        out[:, fo:fo + 128] = np.where((dd >= 0) & (dd <= 6),
                                       w[np.clip(dd, 0, 6)], 0.0)
        out[0:6, fo + 128:fo + 256] = np.where((ww >= 0) & (ww <= 6),
                                               w[np.clip(ww, 0, 6)], 0.0)
    return out

def _ceil_div(a, b):
    return -(-a // b)


def build_program(conv_b: np.ndarray, mode: str = MODE):
    """Build + schedule the Tile program.  Returns the Bacc object."""
    nc = bacc.Bacc("TRN2", target_bir_lowering=False, debug=False,
                   enable_asserts=True)

    x_d = nc.dram_tensor("xs", [BC, L0], F32, kind="ExternalInput").ap()
    cb_d = nc.dram_tensor("cb", [128, N_LAYERS], F32, kind="ExternalInput").ap()
    fcw_d = nc.dram_tensor("fcw", [FC_IN, FC_OUT], F32, kind="ExternalInput").ap()
    fcb_d = nc.dram_tensor("fcb", [FC_OUT, 1], F32, kind="ExternalInput").ap()
    id_d = nc.dram_tensor("ident", [128, 128], F32, kind="ExternalInput").ap()
    if mode == "simple":
        bands_d = nc.dram_tensor("bandsf", [128, N_LAYERS * 128], F32,
                                 kind="ExternalInput").ap()
    else:
        bands_d = nc.dram_tensor("bands", [128, N_LAYERS * 256], BF16,
                                 kind="ExternalInput").ap()
    smoke = int(os.environ.get("KERNEL_SMOKE_LAYERS", "0"))
    nle = smoke if smoke > 0 else N_LAYERS
    if smoke:
        out_d = nc.dram_tensor("outT", [128, 256], F32,
                               kind="ExternalOutput").ap()
    else:
        out_d = nc.dram_tensor("outT", [FC_OUT, BC], F32,
                               kind="ExternalOutput").ap()

    nc0 = _ceil_div(L0, 128)  # 11

    with tile.TileContext(nc) as tc, ExitStack() as ctx:
        cpool = ctx.enter_context(tc.tile_pool(name="const", bufs=1))
        hpool = ctx.enter_context(tc.tile_pool(name="h", bufs=3))
        pspool = ctx.enter_context(tc.tile_pool(name="ps", bufs=6, space="PSUM"))

        xs = cpool.tile([128, L0], F32, tag="xs")
        ident = cpool.tile([128, 128], F32, tag="ident")
        cb_s = cpool.tile([128, N_LAYERS], F32, tag="cb")
        fca = cpool.tile([128, FC_OUT], F32, tag="fca")
        fcb60 = cpool.tile([FC_IN - 128, FC_OUT], F32, tag="fcb60")
        fcbias = cpool.tile([FC_OUT, 1], F32, tag="fcbias")
        outs = cpool.tile([FC_OUT, BC], F32, tag="outs")

        nc.sync.dma_start(xs[:, :], x_d[:, :])
        nc.sync.dma_start(ident[:, :], id_d[:, :])
        nc.sync.dma_start(cb_s[:, :], cb_d[:, :])
        nc.sync.dma_start(fca[:, :], fcw_d[0:128, :])
        nc.sync.dma_start(fcb60[:, :], fcw_d[128:FC_IN, :])
        nc.sync.dma_start(fcbias[:, :], fcb_d[:, :])

        layers_per_chunk = _ceil_div(N_LAYERS, BAND_CHUNKS)
        bw = layers_per_chunk * 256
        bchunks = []
        for i in range(BAND_CHUNKS):
            t = cpool.tile([128, bw], BF16, tag=f"bands{i}")
            nc.sync.dma_start(t[:, :], bands_d[:, bw * i:bw * (i + 1)])
            bchunks.append(t)

        # ---- load x and transpose into H0 (L on partitions) ----
        h0 = hpool.tile([128, nc0 * 128], BF16, tag="h")
        nc.gpsimd.memset(h0[:, 128 * (nc0 - 1):], 0.0)  # finite tail
        cnt = 0
        for c in range(nc0):
            w = min(128, L0 - 128 * c)
            pt = pspool.tile([128, 128], F32, tag="ps")
            nc.tensor.transpose(pt[0:w, :], xs[:, 128 * c:128 * c + w],
                                ident[:, :])
            if cnt % 2 == 0:
                nc.scalar.activation(h0[0:w, 128 * c:128 * c + 128],
                                     pt[0:w, :], AFT.Identity, bias=0.0,
                                     scale=1.0)
            else:
                nc.vector.tensor_copy(h0[0:w, 128 * c:128 * c + 128],
                                      pt[0:w, :])
            cnt += 1

        # ---- 200 conv layers ----
        hin = h0
        Lc = L0
        for l in range(nle):
            Lo = Lc - 6
            nci = _ceil_div(Lc, 128)
            nco = _ceil_div(Lo, 128)
            hout = hpool.tile([128, nco * 128],
                              BF16 if l < N_LAYERS - 1 else F32, tag="h")

            if mode == "simple":
                ch = bchunks[l // layers_per_chunk]
                fo = 128 * (l % layers_per_chunk)
                stF = ch[0:128, fo:fo + 128]
                stW = None  # wrap handled from stacked layout only
                # simple mode still needs the E corner for the wrap matmul:
                # reuse the full band's top-right 6x32 corner? Not available.
                # -> simple mode computes the wrap from the full band's
                #    rows 0:6, cols 96:128 equivalent is NOT present there,
                #    so build the wrap lhsT from the D band layout: the wrap
                #    weights equal w[32+j-r] on rows 0:6, cols 26:32 of a
                #    32-block; the same values appear in the full band at
                #    [j, r] = [j, 96+rr] with j-(96+rr) = ... not present.
                raise NotImplementedError(
                    "simple mode needs the stacked bands input as well")
            else:
                ch = bchunks[l // layers_per_chunk]
                fo = 256 * (l % layers_per_chunk)
                bandf = ch[:, fo:fo + 128]
                wrapf = ch[0:6, fo + 128:fo + 256]

            bval = float(conv_b[l])
            c0 = 0
            while c0 < nco:
                c1 = min(c0 + 4, nco)
                N = (c1 - c0) * 128
                q0 = 128 * c0
                ps = pspool.tile([128, N], F32, tag="ps")
                wn = (min(c1, nci - 1) - c0) * 128

                nc.tensor.matmul(ps[:, 0:N], bandf,
                                 hin[:, q0:q0 + N],
                                 start=True, stop=(wn <= 0),
                                 skip_group_check=True)
                if wn > 0:
                    nc.tensor.matmul(ps[:, 0:wn], wrapf,
                                     hin[0:6, q0 + 128:q0 + 128 + wn],
                                     start=False, stop=True,
                                     skip_group_check=True)

                if l < nle - 1 or smoke:
                    if cnt % 2 == 0:
                        nc.scalar.activation(hout[:, q0:q0 + N], ps[:, 0:N],
                                             AFT.Relu, bias=cb_s[:, l:l + 1],
                                             scale=1.0)
                    else:
                        nc.vector.tensor_scalar(hout[:, q0:q0 + N],
                                                ps[:, 0:N], bval, 0.0,
                                                op0=ALU.add, op1=ALU.max)
                else:
                    nc.vector.tensor_scalar(hout[:, q0:q0 + N], ps[:, 0:N],
                                            bval, None, op0=ALU.add)
                cnt += 1
                c0 = c1

            hin = hout
            Lc = Lo

        if smoke:
            sout = cpool.tile([128, 256], F32, tag="souts")
            nc.scalar.activation(sout[:, :], hin[:, 0:256], AFT.Identity,
                                 bias=0.0, scale=1.0)
            nc.sync.dma_start(out_d[:, :], sout[:, :])
        else:
            # ---- FC 188 -> 91 + sigmoid ----
            assert Lc == FC_IN
            fps = pspool.tile([FC_OUT, BC], F32, tag="ps")
            nc.tensor.matmul(fps[:, :], fca[0:128, :],
                             hin[:, 0:128], start=True, stop=False)
            nc.tensor.matmul(fps[:, :], fcb60[0:FC_IN - 128, :],
                             hin[0:FC_IN - 128, 128:256],
                             start=False, stop=True)
            nc.scalar.activation(outs[:, :], fps[:, :], AFT.Sigmoid,
                                 bias=fcbias[0:FC_OUT, 0:1], scale=1.0)
            nc.sync.dma_start(out_d[:, :], outs[:, :])

    nc.compile()
    return nc


def make_in_maps(x, conv_w, conv_b, fc_w, fc_b, mode: str = MODE):
    x = np.ascontiguousarray(x, np.float32)
    import ml_dtypes
    bands = np.ascontiguousarray(
        _make_bands2(np.asarray(conv_w, np.float32)).astype(
            ml_dtypes.bfloat16))
    cb = np.ascontiguousarray(
        np.broadcast_to(np.asarray(conv_b, np.float32)[None, :],
                        (128, N_LAYERS)))
    fcw = np.ascontiguousarray(np.asarray(fc_w, np.float32).T)  # [188, 91]
    fcb = np.ascontiguousarray(np.asarray(fc_b, np.float32)[:, None])
    ident = np.eye(128, dtype=np.float32)
    bname = "bandsf" if mode == "simple" else "bands"
    in_maps = []
    for i in range(N_CORES):
        in_maps.append({
            "xs": np.ascontiguousarray(x[BC * i:BC * (i + 1)]),
            bname: bands,
            "cb": cb,
            "fcw": fcw,
            "fcb": fcb,
            "ident": ident,
        })
    return in_maps


def run(x, conv_w, conv_b, fc_w, fc_b, mode: str = MODE, **spmd_kwargs):
    nc = build_program(np.asarray(conv_b, np.float32), mode)
    in_maps = make_in_maps(x, conv_w, conv_b, fc_w, fc_b, mode)
    res = run_bass_kernel_spmd(nc, in_maps, list(range(N_CORES)),
                               **spmd_kwargs)
    out = np.concatenate([r["outT"].T for r in res.results], axis=0)
    return np.ascontiguousarray(out, np.float32), res


def kernel(x, conv_w, conv_b, fc_w, fc_b):
    out, _ = run(x, conv_w, conv_b, fc_w, fc_b)
    return out


if __name__ == "__main__":
    rng = np.random.default_rng(0)
    x = rng.normal(size=(1024, L0)).astype(np.float32)
    s = 1.0 / np.sqrt(K7)
    cw = rng.uniform(-s, s, (N_LAYERS, K7)).astype(np.float32)
    cb = rng.uniform(-s, s, N_LAYERS).astype(np.float32)
    sf = 1.0 / np.sqrt(FC_IN)
    fw = rng.uniform(-sf, sf, (FC_OUT, FC_IN)).astype(np.float32)
    fb = rng.uniform(-sf, sf, FC_OUT).astype(np.float32)
    out = kernel(x, cw, cb, fw, fb)
    print(out.shape, out.dtype)



# revision 3
# speedup vs baseline: 1.2854x; 1.2854x over previous
"""Trainium2 Bass kernel for the 200-layer 1-channel Conv1d(k=7) chain + FC + sigmoid.

Strategy (pure data parallel, 8 cores, batch 1024 -> 128/core):
  - On-chip layout keeps the sequence dim on SBUF partitions, interleaved mod 128:
      H[p, 128*c + b] = h[b, 128*c + p]
    so each conv layer is a banded matmul contracting over partitions.
  - The 7-tap band is expressed per layer as a [64, 32] stacked weight block
    (D = within-32-group band, E = cross-group band).  Each 512-wide PSUM block
    is computed by 5 concurrent PE matmuls on disjoint 32x32 sub-array tiles:
      3x (K=64 combined D+E), 1x (K=32 D at row-group 3), 1x (K=6 column-wrap).
  - float32r matmuls (full PE rate at N>=256); relu+bias applied on the
    PSUM->SBUF copy, alternating between the Scalar (ACT) and Vector (DVE)
    engines so neither becomes the serial bottleneck.
  - x is DMA'd naturally and transposed on-chip through the PE (f32).
  - Final Linear(188->91) runs as two accumulating matmuls + fused Sigmoid.
"""

import os
import sys

if "/opt/trn_rl_repo" not in sys.path:
    sys.path.insert(0, "/opt/trn_rl_repo")

from contextlib import ExitStack

import numpy as np

import concourse.bacc as bacc
import concourse.bass as bass
import concourse.mybir as mybir
from concourse import tile
from concourse.bass_utils import run_bass_kernel_spmd

N_CORES = 8
BC = 128          # batch per core
L0 = 1388
N_LAYERS = 200
K7 = 7
FC_IN = 188
FC_OUT = 91

F32 = mybir.dt.float32
F32R = mybir.dt.float32r
BF16 = mybir.dt.bfloat16
AFT = mybir.ActivationFunctionType
ALU = mybir.AluOpType

MODE = "packed32"         # "packed64" | "packed32" | "simple"
BAND_CHUNKS = 4           # weight DMA prefetch chunks (50 layers each)


def _make_bands(conv_w: np.ndarray) -> np.ndarray:
    """[128, 200*64] f32.  Layer l occupies free cols [64l, 64l+64):
    cols [64l, 64l+32)  = D (D[j,r]=w[j-r]) replicated in all four
                          32-partition groups (weights must share the rhs
                          base partition);
    cols [64l+32, 64l+64) = E (E[j,r]=w[32+j-r], rows 0:6) replicated at
                          partition bases 0/32/64/96."""
    out = np.zeros((128, N_LAYERS * 64), np.float32)
    j = np.arange(32)[:, None]
    r = np.arange(32)[None, :]
    dd = j - r            # D taps at 0..6
    ee = 32 + j - r       # E taps at 0..6
    for l in range(N_LAYERS):
        w = conv_w[l]
        D = np.where((dd >= 0) & (dd <= 6), w[np.clip(dd, 0, 6)], 0.0)
        E = np.where((ee >= 0) & (ee <= 6), w[np.clip(ee, 0, 6)], 0.0)
        fo = 64 * l
        for g in range(4):
            out[32 * g:32 * g + 32, fo:fo + 32] = D
            out[32 * g:32 * g + 6, fo + 32:fo + 64] = E[0:6]
    return out


def _make_bands_full(conv_w: np.ndarray) -> np.ndarray:
    """[128, 200*128] f32: per layer the full 128x128 within-column band."""
    out = np.zeros((128, N_LAYERS * 128), np.float32)
    j = np.arange(128)[:, None]
    r = np.arange(128)[None, :]
    dd = j - r
    for l in range(N_LAYERS):
        w = conv_w[l]
        out[:, 128 * l:128 * (l + 1)] = np.where(
            (dd >= 0) & (dd <= 6), w[np.clip(dd, 0, 6)], 0.0)
    return out




def _make_bands2(conv_w: np.ndarray) -> np.ndarray:
    """[128, 200*256]: cols [256l,256l+128) full within-column band
    (B[j,r]=w[j-r], 0<=j-r<=6); cols [256l+128,256l+256) rows 0:6 wrap
    (W[j,r]=w[128+j-r], nonzero r>=122)."""
    out = np.zeros((128, N_LAYERS * 256), np.float32)
    j = np.arange(128)[:, None]
    r = np.arange(128)[None, :]
    dd = j - r
    j6 = np.arange(6)[:, None]
    ww = 128 + j6 - r
    for l in range(N_LAYERS):
        w = conv_w[l]
        fo = 256 * l
        out[:, fo:fo + 128] = np.where((dd >= 0) & (dd <= 6),
                                       w[np.clip(dd, 0, 6)], 0.0)
        out[0:6, fo + 128:fo + 256] = np.where((ww >= 0) & (ww <= 6),
                                               w[np.clip(ww, 0, 6)], 0.0)
    return out

def _ceil_div(a, b):
    return -(-a // b)


def build_program(conv_b: np.ndarray, mode: str = MODE):
    """Build + schedule the Tile program.  Returns the Bacc object."""
    nc = bacc.Bacc("TRN2", target_bir_lowering=False, debug=False,
                   enable_asserts=True)

    x_d = nc.dram_tensor("xs", [BC, L0], F32, kind="ExternalInput").ap()
    cb_d = nc.dram_tensor("cb", [128, N_LAYERS], F32, kind="ExternalInput").ap()
    fcw_d = nc.dram_tensor("fcw", [FC_IN, FC_OUT], F32, kind="ExternalInput").ap()
    fcb_d = nc.dram_tensor("fcb", [FC_OUT, 1], F32, kind="ExternalInput").ap()
    id_d = nc.dram_tensor("ident", [128, 128], F32, kind="ExternalInput").ap()
    if mode == "simple":
        bands_d = nc.dram_tensor("bandsf", [128, N_LAYERS * 128], F32,
                                 kind="ExternalInput").ap()
    else:
        bands_d = nc.dram_tensor("bands", [128, N_LAYERS * 256], BF16,
                                 kind="ExternalInput").ap()
    smoke = int(os.environ.get("KERNEL_SMOKE_LAYERS", "0"))
    nle = smoke if smoke > 0 else N_LAYERS
    if smoke:
        out_d = nc.dram_tensor("outT", [128, 256], F32,
                               kind="ExternalOutput").ap()
    else:
        out_d = nc.dram_tensor("outT", [FC_OUT, BC], F32,
                               kind="ExternalOutput").ap()

    nc0 = _ceil_div(L0, 128)  # 11

    with tile.TileContext(nc) as tc, ExitStack() as ctx:
        cpool = ctx.enter_context(tc.tile_pool(name="const", bufs=1))
        hpool = ctx.enter_context(tc.tile_pool(name="h", bufs=3))
        pspool = ctx.enter_context(tc.tile_pool(name="ps", bufs=6, space="PSUM"))

        xs = cpool.tile([128, L0], F32, tag="xs")
        ident = cpool.tile([128, 128], F32, tag="ident")
        cb_s = cpool.tile([128, N_LAYERS], F32, tag="cb")
        fca = cpool.tile([128, FC_OUT], F32, tag="fca")
        fcb60 = cpool.tile([FC_IN - 128, FC_OUT], F32, tag="fcb60")
        fcbias = cpool.tile([FC_OUT, 1], F32, tag="fcbias")
        outs = cpool.tile([FC_OUT, BC], F32, tag="outs")

        nc.sync.dma_start(xs[:, :], x_d[:, :])
        nc.sync.dma_start(ident[:, :], id_d[:, :])
        nc.sync.dma_start(cb_s[:, :], cb_d[:, :])
        nc.sync.dma_start(fca[:, :], fcw_d[0:128, :])
        nc.sync.dma_start(fcb60[:, :], fcw_d[128:FC_IN, :])
        nc.sync.dma_start(fcbias[:, :], fcb_d[:, :])

        layers_per_chunk = _ceil_div(N_LAYERS, BAND_CHUNKS)
        bw = layers_per_chunk * 256
        bchunks = []
        for i in range(BAND_CHUNKS):
            t = cpool.tile([128, bw], BF16, tag=f"bands{i}")
            nc.sync.dma_start(t[:, :], bands_d[:, bw * i:bw * (i + 1)])
            bchunks.append(t)

        # ---- load x and transpose into H0 (L on partitions) ----
        h0 = hpool.tile([128, nc0 * 128], BF16, tag="h")
        nc.gpsimd.memset(h0[:, 128 * (nc0 - 1):], 0.0)  # finite tail
        cnt = 0
        for c in range(nc0):
            w = min(128, L0 - 128 * c)
            pt = pspool.tile([128, 128], F32, tag="ps")
            nc.tensor.transpose(pt[0:w, :], xs[:, 128 * c:128 * c + w],
                                ident[:, :])
            if cnt % 2 == 0:
                nc.scalar.activation(h0[0:w, 128 * c:128 * c + 128],
                                     pt[0:w, :], AFT.Identity, bias=0.0,
                                     scale=1.0)
            else:
                nc.vector.tensor_copy(h0[0:w, 128 * c:128 * c + 128],
                                      pt[0:w, :])
            cnt += 1

        # ---- 200 conv layers ----
        hin = h0
        Lc = L0
        for l in range(nle):
            Lo = Lc - 6
            nci = _ceil_div(Lc, 128)
            nco = _ceil_div(Lo, 128)
            hout = hpool.tile([128, nco * 128],
                              BF16 if l < N_LAYERS - 1 else F32, tag="h")

            if mode == "simple":
                ch = bchunks[l // layers_per_chunk]
                fo = 128 * (l % layers_per_chunk)
                stF = ch[0:128, fo:fo + 128]
                stW = None  # wrap handled from stacked layout only
                # simple mode still needs the E corner for the wrap matmul:
                # reuse the full band's top-right 6x32 corner? Not available.
                # -> simple mode computes the wrap from the full band's
                #    rows 0:6, cols 96:128 equivalent is NOT present there,
                #    so build the wrap lhsT from the D band layout: the wrap
                #    weights equal w[32+j-r] on rows 0:6, cols 26:32 of a
                #    32-block; the same values appear in the full band at
                #    [j, r] = [j, 96+rr] with j-(96+rr) = ... not present.
                raise NotImplementedError(
                    "simple mode needs the stacked bands input as well")
            else:
                ch = bchunks[l // layers_per_chunk]
                fo = 256 * (l % layers_per_chunk)
                bandf = ch[:, fo:fo + 128]
                # full-128 K for the wrap: rows 6..127 are zero in the bands
                # layout, and keeping rhs partition size at 128 keeps the PE
                # tile_size at (128,128) for every matmul — mixing (32,128)
                # wraps with (128,128) mains forces a PE tiling-mode drain
                # between every pair.
                wrapf = ch[:, fo + 128:fo + 256]

            bval = float(conv_b[l])
            c0 = 0
            while c0 < nco:
                c1 = min(c0 + 4, nco)
                N = (c1 - c0) * 128
                q0 = 128 * c0
                ps = pspool.tile([128, N], F32, tag="ps")
                wn = (min(c1, nci - 1) - c0) * 128

                nc.tensor.matmul(ps[:, 0:N], bandf,
                                 hin[:, q0:q0 + N],
                                 start=True, stop=(wn <= 0),
                                 skip_group_check=True)
                if wn > 0:
                    nc.tensor.matmul(ps[:, 0:wn], wrapf,
                                     hin[:, q0 + 128:q0 + 128 + wn],
                                     start=False, stop=True,
                                     skip_group_check=True)

                if l < nle - 1 or smoke:
                    if cnt % 2 == 0:
                        nc.scalar.activation(hout[:, q0:q0 + N], ps[:, 0:N],
                                             AFT.Relu, bias=cb_s[:, l:l + 1],
                                             scale=1.0)
                    else:
                        nc.vector.tensor_scalar(hout[:, q0:q0 + N],
                                                ps[:, 0:N], bval, 0.0,
                                                op0=ALU.add, op1=ALU.max)
                else:
                    nc.vector.tensor_scalar(hout[:, q0:q0 + N], ps[:, 0:N],
                                            bval, None, op0=ALU.add)
                cnt += 1
                c0 = c1

            hin = hout
            Lc = Lo

        if smoke:
            sout = cpool.tile([128, 256], F32, tag="souts")
            nc.scalar.activation(sout[:, :], hin[:, 0:256], AFT.Identity,
                                 bias=0.0, scale=1.0)
            nc.sync.dma_start(out_d[:, :], sout[:, :])
        else:
            # ---- FC 188 -> 91 + sigmoid ----
            assert Lc == FC_IN
            fps = pspool.tile([FC_OUT, BC], F32, tag="ps")
            nc.tensor.matmul(fps[:, :], fca[0:128, :],
                             hin[:, 0:128], start=True, stop=False)
            nc.tensor.matmul(fps[:, :], fcb60[0:FC_IN - 128, :],
                             hin[0:FC_IN - 128, 128:256],
                             start=False, stop=True)
            nc.scalar.activation(outs[:, :], fps[:, :], AFT.Sigmoid,
                                 bias=fcbias[0:FC_OUT, 0:1], scale=1.0)
            nc.sync.dma_start(out_d[:, :], outs[:, :])

    nc.compile()
    return nc


def make_in_maps(x, conv_w, conv_b, fc_w, fc_b, mode: str = MODE):
    x = np.ascontiguousarray(x, np.float32)
    import ml_dtypes
    bands = np.ascontiguousarray(
        _make_bands2(np.asarray(conv_w, np.float32)).astype(
            ml_dtypes.bfloat16))
    cb = np.ascontiguousarray(
        np.broadcast_to(np.asarray(conv_b, np.float32)[None, :],
                        (128, N_LAYERS)))
    fcw = np.ascontiguousarray(np.asarray(fc_w, np.float32).T)  # [188, 91]
    fcb = np.ascontiguousarray(np.asarray(fc_b, np.float32)[:, None])
    ident = np.eye(128, dtype=np.float32)
    bname = "bandsf" if mode == "simple" else "bands"
    in_maps = []
    for i in range(N_CORES):
        in_maps.append({
            "xs": np.ascontiguousarray(x[BC * i:BC * (i + 1)]),
            bname: bands,
            "cb": cb,
            "fcw": fcw,
            "fcb": fcb,
            "ident": ident,
        })
    return in_maps


def run(x, conv_w, conv_b, fc_w, fc_b, mode: str = MODE, **spmd_kwargs):
    nc = build_program(np.asarray(conv_b, np.float32), mode)
    in_maps = make_in_maps(x, conv_w, conv_b, fc_w, fc_b, mode)
    res = run_bass_kernel_spmd(nc, in_maps, list(range(N_CORES)),
                               **spmd_kwargs)
    out = np.concatenate([r["outT"].T for r in res.results], axis=0)
    return np.ascontiguousarray(out, np.float32), res


def kernel(x, conv_w, conv_b, fc_w, fc_b):
    out, _ = run(x, conv_w, conv_b, fc_w, fc_b)
    return out


if __name__ == "__main__":
    rng = np.random.default_rng(0)
    x = rng.normal(size=(1024, L0)).astype(np.float32)
    s = 1.0 / np.sqrt(K7)
    cw = rng.uniform(-s, s, (N_LAYERS, K7)).astype(np.float32)
    cb = rng.uniform(-s, s, N_LAYERS).astype(np.float32)
    sf = 1.0 / np.sqrt(FC_IN)
    fw = rng.uniform(-sf, sf, (FC_OUT, FC_IN)).astype(np.float32)
    fb = rng.uniform(-sf, sf, FC_OUT).astype(np.float32)
    out = kernel(x, cw, cb, fw, fb)
    print(out.shape, out.dtype)



# revision 4
# speedup vs baseline: 1.9988x; 1.5550x over previous
"""Trainium2 Bass kernel for the 200-layer 1-channel Conv1d(k=7) chain + FC + sigmoid.

Strategy (pure data parallel, 8 cores, batch 1024 -> 128/core):
  - On-chip layout keeps the sequence dim on SBUF partitions, interleaved
    mod 128, with the 128-batch split into TWO independent 64-col halves:
      H[p, s*HW + 64*c + b] = h[64*s + b, 128*c + p]
    Each conv layer is a banded matmul (within-chunk band, K=128) plus a
    chunk-boundary wrap matmul (K padded to 128 so every matmul keeps the
    same PE tile_size -- mixed sizes force a PE tiling-mode drain).
  - The two batch halves form independent dependency chains; interleaving
    them at group granularity hides each half's PSUM-evacuation latency
    under the other half's matmuls, keeping the PE continuously busy (and
    therefore ramped to its max p-state clock).
  - Chunks are processed top-down (descending), with a small 3-chunk head
    group per layer: the next layer's head depends only on this head's
    evacuation, which minimizes the serial layer-to-layer critical path.
  - relu+bias fused into the PSUM->SBUF copy, alternating Scalar (ACT) and
    Vector (DVE) engines.
  - x is DMA'd naturally and transposed on-chip through the PE (f32).
  - Final Linear(188->91) runs as accumulating matmuls + fused Sigmoid.
"""

import sys

if "/opt/trn_rl_repo" not in sys.path:
    sys.path.insert(0, "/opt/trn_rl_repo")

from contextlib import ExitStack

import numpy as np

import concourse.bacc as bacc
import concourse.mybir as mybir
from concourse import tile
from concourse.bass_utils import run_bass_kernel_spmd

N_CORES = 8
BC = 128          # batch per core
HALF = 64         # batch cols per half-pipeline
L0 = 1388
N_LAYERS = 200
K7 = 7
FC_IN = 188
FC_OUT = 91
NC0 = -(-L0 // 128)          # 11 chunks
HW = NC0 * HALF              # per-half H width in cols

F32 = mybir.dt.float32
BF16 = mybir.dt.bfloat16
AFT = mybir.ActivationFunctionType
ALU = mybir.AluOpType

# weight DMA prefetch chunk sizes (layers per chunk; first ones small so
# layer 0 is not gated on a multi-MB transfer)
WCHUNKS = [2, 3, 5, 8, 12] + [17] * 10


def _make_bands2(conv_w: np.ndarray) -> np.ndarray:
    """[128, 200*256]: cols [256l,256l+128) full within-column band
    (B[j,r]=w[j-r], 0<=j-r<=6); cols [256l+128,256l+256) rows 0:6 wrap
    (W[j,r]=w[128+j-r], nonzero r>=122)."""
    out = np.zeros((128, N_LAYERS * 256), np.float32)
    j = np.arange(128)[:, None]
    r = np.arange(128)[None, :]
    dd = j - r
    j6 = np.arange(6)[:, None]
    ww = 128 + j6 - r
    for l in range(N_LAYERS):
        w = conv_w[l]
        fo = 256 * l
        out[:, fo:fo + 128] = np.where((dd >= 0) & (dd <= 6),
                                       w[np.clip(dd, 0, 6)], 0.0)
        out[0:6, fo + 128:fo + 256] = np.where((ww >= 0) & (ww <= 6),
                                               w[np.clip(ww, 0, 6)], 0.0)
    return out


def _ceil_div(a, b):
    return -(-a // b)


def _layer_groups(nco):
    """Descending chunk groups: small head (top 3 chunks) first, then
    groups of up to 8 below it."""
    k = min(3, nco)
    groups = [(nco - k, k)]
    c = nco - k
    while c > 0:
        n = min(8, c)
        groups.append((c - n, n))
        c -= n
    return groups


def build_program(conv_b: np.ndarray):
    nc = bacc.Bacc("TRN2", target_bir_lowering=False, debug=False,
                   enable_asserts=True)

    x_d = nc.dram_tensor("xs", [BC, L0], F32, kind="ExternalInput").ap()
    cb_d = nc.dram_tensor("cb", [128, N_LAYERS], F32, kind="ExternalInput").ap()
    fcw_d = nc.dram_tensor("fcw", [FC_IN, FC_OUT], F32, kind="ExternalInput").ap()
    fcb_d = nc.dram_tensor("fcb", [FC_OUT, 1], F32, kind="ExternalInput").ap()
    id_d = nc.dram_tensor("ident", [128, 128], F32, kind="ExternalInput").ap()
    bands_d = nc.dram_tensor("bands", [128, N_LAYERS * 256], BF16,
                             kind="ExternalInput").ap()
    out_d = nc.dram_tensor("outT", [FC_OUT, BC], F32, kind="ExternalOutput").ap()

    with tile.TileContext(nc) as tc, ExitStack() as ctx:
        cpool = ctx.enter_context(tc.tile_pool(name="const", bufs=1))
        hpool = ctx.enter_context(tc.tile_pool(name="h", bufs=3))
        pspool = ctx.enter_context(tc.tile_pool(name="ps", bufs=6, space="PSUM"))

        xs = cpool.tile([128, L0], F32, tag="xs")
        ident = cpool.tile([128, 128], F32, tag="ident")
        cb_s = cpool.tile([128, N_LAYERS], F32, tag="cb")
        fca = cpool.tile([128, FC_OUT], F32, tag="fca")
        fcb60 = cpool.tile([FC_IN - 128, FC_OUT], F32, tag="fcb60")
        fcbias = cpool.tile([FC_OUT, 1], F32, tag="fcbias")
        outs = cpool.tile([FC_OUT, BC], F32, tag="outs")

        nc.sync.dma_start(xs[:, :], x_d[:, :])
        nc.sync.dma_start(ident[:, :], id_d[:, :])
        nc.sync.dma_start(cb_s[:, :], cb_d[:, :])
        nc.sync.dma_start(fca[:, :], fcw_d[0:128, :])
        nc.sync.dma_start(fcb60[:, :], fcw_d[128:FC_IN, :])
        nc.sync.dma_start(fcbias[:, :], fcb_d[:, :])

        bchunks = []   # (tile, first_layer, n_layers)
        l0 = 0
        for n in WCHUNKS:
            if l0 >= N_LAYERS:
                break
            n = min(n, N_LAYERS - l0)
            t = cpool.tile([128, n * 256], BF16, tag=f"bands{l0}")
            nc.sync.dma_start(t[:, :], bands_d[:, 256 * l0:256 * (l0 + n)])
            bchunks.append((t, l0, n))
            l0 += n
        assert l0 == N_LAYERS

        def band_slices(l):
            for t, a, n in bchunks:
                if a <= l < a + n:
                    fo = 256 * (l - a)
                    return t[:, fo:fo + 128], t[:, fo + 128:fo + 256]
            raise AssertionError

        # ---- load x and transpose into H0 (positions on partitions,
        #      two 64-col batch halves side by side) ----
        h0 = hpool.tile([128, 2 * HW], BF16, tag="h")
        # finite tail of the last chunk (partitions >= L0-128*(NC0-1))
        nc.gpsimd.memset(h0[:, (NC0 - 1) * HALF:NC0 * HALF], 0.0)
        nc.gpsimd.memset(h0[:, HW + (NC0 - 1) * HALF:HW + NC0 * HALF], 0.0)
        cnt = 0
        for c in range(NC0):
            w = min(128, L0 - 128 * c)
            pt = pspool.tile([128, 512], F32, tag="ps")
            nc.tensor.transpose(pt[0:w, 0:128], xs[:, 128 * c:128 * c + w],
                                ident[:, :])
            for s in range(2):
                dst = h0[0:w, s * HW + c * HALF:s * HW + (c + 1) * HALF]
                src = pt[0:w, s * HALF:(s + 1) * HALF]
                if cnt % 2 == 0:
                    nc.scalar.activation(dst, src, AFT.Identity, bias=0.0,
                                         scale=1.0)
                else:
                    nc.vector.tensor_copy(dst, src)
                cnt += 1

        # ---- 200 conv layers ----
        hin = h0
        Lc = L0
        for l in range(N_LAYERS):
            Lo = Lc - 6
            nci = _ceil_div(Lc, 128)
            nco = _ceil_div(Lo, 128)
            last = l == N_LAYERS - 1
            hout = hpool.tile([128, 2 * HW], F32 if last else BF16, tag="h")
            bandf, wrapf = band_slices(l)
            bval = float(conv_b[l])

            for c0, nch in _layer_groups(nco):
                for s in range(2):
                    N = nch * HALF
                    q = s * HW + c0 * HALF
                    ps = pspool.tile([128, 512], F32, tag="ps")
                    wn_ch = min(c0 + nch, nci - 1) - c0
                    nc.tensor.matmul(ps[:, 0:N], bandf, hin[:, q:q + N],
                                     start=True, stop=(wn_ch <= 0),
                                     skip_group_check=True)
                    if wn_ch > 0:
                        wn = wn_ch * HALF
                        nc.tensor.matmul(ps[:, 0:wn], wrapf,
                                         hin[:, q + HALF:q + HALF + wn],
                                         start=False, stop=True,
                                         skip_group_check=True)
                    dst = hout[:, q:q + N]
                    if last:
                        nc.vector.tensor_scalar(dst, ps[:, 0:N], bval, None,
                                                op0=ALU.add)
                    elif cnt % 2 == 0:
                        nc.scalar.activation(dst, ps[:, 0:N], AFT.Relu,
                                             bias=cb_s[:, l:l + 1], scale=1.0)
                    else:
                        nc.vector.tensor_scalar(dst, ps[:, 0:N], bval, 0.0,
                                                op0=ALU.add, op1=ALU.max)
                    cnt += 1

            hin = hout
            Lc = Lo

        # ---- FC 188 -> 91 + sigmoid ----
        assert Lc == FC_IN
        fpt = pspool.tile([128, 512], F32, tag="ps")
        fps = fpt[0:FC_OUT, 0:BC]
        for s in range(2):
            nc.tensor.matmul(fps[:, s * HALF:(s + 1) * HALF], fca[0:128, :],
                             hin[:, s * HW:s * HW + HALF],
                             start=True, stop=False, skip_group_check=True)
            nc.tensor.matmul(fps[:, s * HALF:(s + 1) * HALF],
                             fcb60[0:FC_IN - 128, :],
                             hin[0:FC_IN - 128, s * HW + HALF:s * HW + 2 * HALF],
                             start=False, stop=True, skip_group_check=True)
        nc.scalar.activation(outs[:, :], fps[:, :], AFT.Sigmoid,
                             bias=fcbias[0:FC_OUT, 0:1], scale=1.0)
        nc.sync.dma_start(out_d[:, :], outs[:, :])

    nc.compile()
    return nc


def make_in_maps(x, conv_w, conv_b, fc_w, fc_b):
    x = np.ascontiguousarray(x, np.float32)
    import ml_dtypes
    bands = np.ascontiguousarray(
        _make_bands2(np.asarray(conv_w, np.float32)).astype(
            ml_dtypes.bfloat16))
    cb = np.ascontiguousarray(
        np.broadcast_to(np.asarray(conv_b, np.float32)[None, :],
                        (128, N_LAYERS)))
    fcw = np.ascontiguousarray(np.asarray(fc_w, np.float32).T)  # [188, 91]
    fcb = np.ascontiguousarray(np.asarray(fc_b, np.float32)[:, None])
    ident = np.eye(128, dtype=np.float32)
    in_maps = []
    for i in range(N_CORES):
        in_maps.append({
            "xs": np.ascontiguousarray(x[BC * i:BC * (i + 1)]),
            "bands": bands,
            "cb": cb,
            "fcw": fcw,
            "fcb": fcb,
            "ident": ident,
        })
    return in_maps


def run(x, conv_w, conv_b, fc_w, fc_b, **spmd_kwargs):
    nc = build_program(np.asarray(conv_b, np.float32))
    in_maps = make_in_maps(x, conv_w, conv_b, fc_w, fc_b)
    res = run_bass_kernel_spmd(nc, in_maps, list(range(N_CORES)),
                               **spmd_kwargs)
    # per-core outT is [91, 128] with batch cols [half0 | half1]; halves are
    # contiguous batch ranges, so a plain transpose restores batch order.
    out = np.concatenate([r["outT"].T for r in res.results], axis=0)
    return np.ascontiguousarray(out, np.float32), res


def kernel(x, conv_w, conv_b, fc_w, fc_b):
    out, _ = run(x, conv_w, conv_b, fc_w, fc_b)
    return out


if __name__ == "__main__":
    rng = np.random.default_rng(0)
    x = rng.normal(size=(1024, L0)).astype(np.float32)
    s = 1.0 / np.sqrt(K7)
    cw = rng.uniform(-s, s, (N_LAYERS, K7)).astype(np.float32)
    cb = rng.uniform(-s, s, N_LAYERS).astype(np.float32)
    sf = 1.0 / np.sqrt(FC_IN)
    fw = rng.uniform(-sf, sf, (FC_OUT, FC_IN)).astype(np.float32)
    fb = rng.uniform(-sf, sf, FC_OUT).astype(np.float32)
    out = kernel(x, cw, cb, fw, fb)
    print(out.shape, out.dtype)
